# revision 10
# baseline (speedup 1.0000x reference)
"""ConvTransE forward on 8 Trainium2 NeuronCores (Bass/Tile) — bf16 PE, v5.

Math shortcut: the reference computes scores = x @ ent.T ([B, 100000]) and
returns scores[i, t[i]]; we only compute out[b] = x[b] . ent[t[b]].
The conv's retained slice [:, :512] depends only on ent[h] and rel[r][:, 0].

Sharding: tensor-parallel over the projection contraction dim (channels).
Core m owns channels [4m, 4m+4).  Every core:
  - gathers ent[h] rows (cast fp32->bf16 in the DMA), rel[r][:, 0] only,
    and ent[t] rows (fp32) via indirect DMA,
  - PE-transposes overlapping 128-wide windows of the gathered rows (bf16),
  - runs the conv as banded matmuls on the PE (bands built on host, bf16),
  - projects its K-slice:  z_m = relu(conv)_m @ proj_w_m^T  (K = 2048/core),
  - emits partial[b] = z_m[b] . ent[t[b]] via a fused multiply+row-sum (fp32).
proj_b rides along as an extra "ones" contraction row on core 0 only.
Host sums the 8 [2048] partials.

v10: chunk 0 conv split into btl-pair halves (starts after 2 h-gathers)
with proj b0/b1 mains interleaved between halves.
v9: band/pwt head load early, only the pwt tail gated; chunk 0 runs
b1's main projection before b0's stub-close so ystub is off the
critical path.
v7: bf16 entity table in DRAM (halves gather HBM reads; t-rows
upcast to fp32 in the DMA), pwt tail loads gated behind the first
h-gathers via WAW dummies, chunk 0 interleaves the head of its own
first projection group into its conv phase.
v6: grouped per-btl projection (no per-matmul PSUM bank cycling — that
measured ~20% slower per matmul), conv PSUM tiles hold 2 segments and
relu+bias runs once per pair (ACT/DVE alternating), and each chunk's
4th projection group is deferred into the middle of the NEXT chunk's
conv phase so the conv relu-drain latency is hidden behind projection
matmuls instead of stalling the in-order PE queue. Gathers prefetched
one chunk ahead (h, then rel, then t), index DMAs before weight DMAs,
pwt in two DMAs, stub transposes packed into each btl's transpose tile.
"""

import numpy as np

NE, NRR, D, C, B = 100000, 500, 512, 32, 2048
NCORES = 8
CPC = C // NCORES          # 4 channels per core
NQ = B // 128              # 16 batch tiles of 128
CHUNK_BT = 4               # batch tiles per pipeline chunk
NCHUNK = NQ // CHUNK_BT
NB = CHUNK_BT * 128        # 512 batch columns per chunk
JB = 126                   # conv j-block (126 outputs need a 128-wide input window)
NSEG = 16                  # (c, s) main contraction blocks per core
KSTUB = CPC * 8 + 1        # 33: packed j=504..511 stub rows + ones row
GWW = 640                  # per-btl transpose tile: 4 windows * 128 + stub 128

_CACHE = {}


def _build_nc():
    from contextlib import ExitStack

    import concourse.bass as bass
    import concourse.tile as tile
    from concourse import bacc, mybir
    from concourse.masks import make_identity

    f32 = mybir.dt.float32
    bf16 = mybir.dt.bfloat16
    i32 = mybir.dt.int32
    Alu = mybir.AluOpType

    nc = bacc.Bacc("TRN2", target_bir_lowering=False, debug=False,
                   num_devices=NCORES)

    ent = nc.dram_tensor("ent", [NE, D], bf16, kind="ExternalInput")
    rel = nc.dram_tensor("rel", [NRR, D], f32, kind="ExternalInput")
    hI = nc.dram_tensor("hI", [128, NQ], i32, kind="ExternalInput")
    tI = nc.dram_tensor("tI", [128, NQ], i32, kind="ExternalInput")
    rI = nc.dram_tensor("rI", [128, NQ], i32, kind="ExternalInput")
    band = nc.dram_tensor("band", [128, NSEG * JB], bf16, kind="ExternalInput")
    bstub = nc.dram_tensor("bstub", [10, 32], bf16, kind="ExternalInput")
    pwt = nc.dram_tensor("pwt", [JB, NSEG * D], bf16, kind="ExternalInput")
    pstub = nc.dram_tensor("pstub", [KSTUB, D], bf16, kind="ExternalInput")
    cbias = nc.dram_tensor("cbias", [128, CPC], f32, kind="ExternalInput")
    sbias = nc.dram_tensor("sbias", [32, 1], f32, kind="ExternalInput")
    out = nc.dram_tensor("out", [128, NQ], f32, kind="ExternalOutput")

    with tile.TileContext(nc) as tc, ExitStack() as ctx:
        const = ctx.enter_context(tc.tile_pool(name="const", bufs=1))
        gpad_p = ctx.enter_context(tc.tile_pool(name="gpad", bufs=10))
        v_p = ctx.enter_context(tc.tile_pool(name="vt", bufs=12))
        gw_p = ctx.enter_context(tc.tile_pool(name="gw", bufs=2))
        gws_p = ctx.enter_context(tc.tile_pool(name="gws", bufs=2))
        y_p = ctx.enter_context(tc.tile_pool(name="ym", bufs=2))
        ys_p = ctx.enter_context(tc.tile_pool(name="ys", bufs=2))
        sc_p = ctx.enter_context(tc.tile_pool(name="scr", bufs=2))
        tp_p = ctx.enter_context(tc.tile_pool(name="tp", bufs=2, space="PSUM"))
        yp_p = ctx.enter_context(tc.tile_pool(name="yp", bufs=2, space="PSUM"))
        z_p = ctx.enter_context(tc.tile_pool(name="zp", bufs=2, space="PSUM"))

        # tiny index tables first: the first gathers depend only on these
        hI_sb = const.tile([128, NQ], i32)
        nc.sync.dma_start(hI_sb[:], hI[:])
        rI_sb = const.tile([128, NQ], i32)
        nc.sync.dma_start(rI_sb[:], rI[:])
        ident = const.tile([128, 128], bf16)
        make_identity(nc, ident[:])
        band_sb = const.tile([128, NSEG * JB], bf16)
        nc.sync.dma_start(band_sb[:], band[:])
        pwt_sb = const.tile([JB, NSEG * D], bf16)
        nc.sync.dma_start(pwt_sb[:, 0:4 * D], pwt[:, 0:4 * D])
        bstub_sb = const.tile([10, 32], bf16)
        nc.sync.dma_start(bstub_sb[:], bstub[:])
        cb_sb = const.tile([128, CPC], f32)
        nc.sync.dma_start(cb_sb[:], cbias[:])
        sb_sb = const.tile([32, 1], f32)
        nc.sync.dma_start(sb_sb[:], sbias[:])
        tI_sb = const.tile([128, NQ], i32)
        nc.sync.dma_start(tI_sb[:], tI[:])
        pstub_sb = const.tile([KSTUB, D], bf16)
        nc.sync.dma_start(pstub_sb[:], pstub[:])
        out_sb = const.tile([128, NQ], f32)

        def emit_gathers(c):
            gpads, vts = [], []
            for btl in range(CHUNK_BT):
                q = c * CHUNK_BT + btl
                gpad = gpad_p.tile([128, D + 2], bf16, name="gpad")
                nc.vector.memset(gpad[:, 0:1], 0.0)
                nc.gpsimd.indirect_dma_start(
                    out=gpad[:, 1:D + 1], out_offset=None, in_=ent[:],
                    in_offset=bass.IndirectOffsetOnAxis(
                        ap=hI_sb[:, q:q + 1], axis=0))
                gpads.append(gpad)
            for btl in range(CHUNK_BT):
                q = c * CHUNK_BT + btl
                nc.gpsimd.indirect_dma_start(
                    out=gpads[btl][:, D + 1:D + 2], out_offset=None,
                    in_=rel[:, 0:1],
                    in_offset=bass.IndirectOffsetOnAxis(
                        ap=rI_sb[:, q:q + 1], axis=0))
            for btl in range(CHUNK_BT):
                q = c * CHUNK_BT + btl
                vt = v_p.tile([128, D], f32, name="vt")
                nc.gpsimd.indirect_dma_start(
                    out=vt[:], out_offset=None, in_=ent[:],
                    in_offset=bass.IndirectOffsetOnAxis(
                        ap=tI_sb[:, q:q + 1], axis=0))
                vts.append(vt)
            return gpads, vts

        def emit_proj_head(q, ym, hi):
            z = z_p.tile([128, D], f32, name="zt")
            b = q % CHUNK_BT
            for i in range(hi):
                nc.tensor.matmul(
                    z[:], ym[:, i * NB + b * 128:i * NB + (b + 1) * 128],
                    pwt_sb[:, i * D:(i + 1) * D],
                    start=(i == 0), stop=False)
            return z

        def emit_proj_main(q, ym, z=None, lo=0):
            b = q % CHUNK_BT
            if z is None:
                z = z_p.tile([128, D], f32, name="zt")
            for i in range(lo, NSEG):
                nc.tensor.matmul(
                    z[:], ym[:, i * NB + b * 128:i * NB + (b + 1) * 128],
                    pwt_sb[:, i * D:(i + 1) * D],
                    start=(i == 0), stop=False)
            return z

        def emit_proj_close(q, z, ystub, vt):
            b = q % CHUNK_BT
            nc.tensor.matmul(z[:], ystub[:, b * 128:(b + 1) * 128],
                             pstub_sb[:], start=False, stop=True)
            scr = sc_p.tile([128, D], f32)
            nc.vector.scalar_tensor_tensor(
                out=scr[:], in0=z[:], scalar=1.0, in1=vt[:],
                op0=Alu.mult, op1=Alu.mult,
                accum_out=out_sb[:, q:q + 1])

        def emit_proj(q, ym, ystub, vt, z=None, lo=0):
            z = emit_proj_main(q, ym, z=z, lo=lo)
            emit_proj_close(q, z, ystub, vt)

        pending = emit_gathers(0)
        # gate the big weight loads on the first h-gathers landing so the
        # chunk-0 gathers aren't starved of HBM by the 2.5MB weight stream
        # (WAW dummy copies order each DMA behind a gather completion)
        nc.vector.tensor_copy(pwt_sb[0:1, 4 * D:4 * D + 1],
                              pending[0][0][0:1, 1:2])
        nc.sync.dma_start(pwt_sb[:, 4 * D:10 * D], pwt[:, 4 * D:10 * D])
        nc.vector.tensor_copy(pwt_sb[0:1, 10 * D:10 * D + 1],
                              pending[0][1][0:1, 1:2])
        nc.sync.dma_start(pwt_sb[:, 10 * D:], pwt[:, 10 * D:])
        deferred = None    # (q, ym, ystub, vt) for the previous chunk's b3
        for chunk in range(NCHUNK):
            gpads, vts = pending
            if chunk + 1 < NCHUNK:
                pending = emit_gathers(chunk + 1)

            # per-btl: 4 main window transposes + stub transpose into one
            # PSUM tile, one main copy into btl-major gw, one stub copy
            gw = gw_p.tile([128, CHUNK_BT * D], bf16)
            gwv = gw[:].rearrange("p (b s c) -> p b s c",
                                  b=CHUNK_BT, s=4, c=128)
            gws = gws_p.tile([10, NB], bf16)
            def emit_transp(btl):
                gpad = gpads[btl]
                tp = tp_p.tile([128, GWW], bf16)
                for s in range(4):
                    nc.tensor.transpose(tp[:, s * 128:(s + 1) * 128],
                                        gpad[:, JB * s:JB * s + 128], ident[:])
                nc.tensor.transpose(tp[0:10, 512:640],
                                    gpad[:, 4 * JB:D + 2], ident[:])
                nc.vector.tensor_copy(gw[:, btl * D:(btl + 1) * D],
                                      tp[:, 0:512])
                nc.vector.tensor_copy(gws[:, btl * 128:(btl + 1) * 128],
                                      tp[0:10, 512:640])

            if chunk > 0:
                for btl in range(CHUNK_BT):
                    emit_transp(btl)

            ym = y_p.tile([JB, NSEG * NB], bf16)
            ystub = ys_p.tile([KSTUB, NB], bf16)
            nc.vector.memset(ystub[32:33, :], 1.0)

            ymv = ym[:].rearrange("p (s b c) -> p s b c",
                                  s=NSEG, b=CHUNK_BT, c=128)

            # chunk 0 only: conv over btl pairs so the PE can start as
            # soon as the first two h-gathers land
            def conv_pair_half(k, half):
                yp = yp_p.tile([JB, 2 * NB], f32, name="yp")
                off = half * NB
                for j in range(2):
                    cs = 2 * k + j
                    nc.tensor.matmul(
                        yp[:, off + j * 256:off + (j + 1) * 256],
                        band_sb[:, cs * JB:(cs + 1) * JB],
                        gwv[:, 2 * half:2 * half + 2, cs % 4, :],
                        start=True, stop=True)
                c4 = (2 * k) // 4
                src_v = yp[:, off:off + 512].rearrange(
                    "p (s b c) -> p s b c", s=2, b=2, c=128)
                dst_v = ymv[:, 2 * k:2 * k + 2, 2 * half:2 * half + 2, :]
                if k % 2 == 0:
                    nc.scalar.activation(
                        dst_v, src_v, mybir.ActivationFunctionType.Relu,
                        bias=cb_sb[0:JB, c4:c4 + 1])
                else:
                    nc.vector.tensor_scalar(dst_v, src_v,
                                            cb_sb[0:JB, c4:c4 + 1],
                                            0.0, Alu.add, Alu.max)

            # conv: 2 segments per PSUM tile, one relu+bias op per pair
            def conv_pair(k):
                yp = yp_p.tile([JB, 2 * NB], f32, name="yp")
                for j in range(2):
                    cs = 2 * k + j
                    s4 = cs % 4
                    nc.tensor.matmul(yp[:, j * NB:(j + 1) * NB],
                                     band_sb[:, cs * JB:(cs + 1) * JB],
                                     gwv[:, :, s4, :], start=True, stop=True)
                c4 = (2 * k) // 4
                if k % 2 == 0:
                    nc.scalar.activation(
                        ym[:, 2 * k * NB:(2 * k + 2) * NB], yp[:],
                        mybir.ActivationFunctionType.Relu,
                        bias=cb_sb[0:JB, c4:c4 + 1])
                else:
                    nc.vector.tensor_scalar(ym[:, 2 * k * NB:(2 * k + 2) * NB],
                                            yp[:], cb_sb[0:JB, c4:c4 + 1],
                                            0.0, Alu.add, Alu.max)

            q0 = chunk * CHUNK_BT
            if deferred is None:
                # chunk 0: [transp b0,b1][conv half A][proj b0 main]
                #          [transp b2,b3][conv half B][proj b1 main]
                #          [stub][closes][proj b2][defer b3]
                emit_transp(0)
                emit_transp(1)
                for k in range(NSEG // 2):
                    conv_pair_half(k, 0)
                z0 = emit_proj_main(q0, ym)
                emit_transp(2)
                emit_transp(3)
                for k in range(NSEG // 2):
                    conv_pair_half(k, 1)
                z1 = emit_proj_main(q0 + 1, ym)
            else:
                for k in range(4):
                    conv_pair(k)
                # previous chunk's 4th projection group fills the PE while
                # this chunk's first relus drain
                emit_proj(*deferred)
                for k in range(4, NSEG // 2):
                    conv_pair(k)
            yps = yp_p.tile([JB, 2 * NB], f32, name="yp")
            nc.tensor.matmul(yps[0:32, 0:NB], bstub_sb[:], gws[:],
                             start=True, stop=True)
            nc.scalar.activation(ystub[0:32, :], yps[0:32, 0:NB],
                                 mybir.ActivationFunctionType.Relu,
                                 bias=sb_sb[:, 0:1])

            if deferred is None:
                emit_proj_close(q0, z0, ystub, vts[0])
                emit_proj_close(q0 + 1, z1, ystub, vts[1])
                emit_proj(q0 + 2, ym, ystub, vts[2])
            else:
                for btl in range(CHUNK_BT - 1):
                    emit_proj(q0 + btl, ym, ystub, vts[btl])
            deferred = (q0 + 3, ym, ystub, vts[3])

        emit_proj(*deferred)
        nc.sync.dma_start(out[:], out_sb[:])
    nc.finalize()
    return nc


def _host_prep(inputs):
    """Per-core input dicts from the full problem inputs."""
    import ml_dtypes

    bf = ml_dtypes.bfloat16
    ent = np.ascontiguousarray(
        np.asarray(inputs["ent"], dtype=np.float32).astype(bf))
    rel = np.ascontiguousarray(np.asarray(inputs["rel"], dtype=np.float32))
    w = np.asarray(inputs["conv_w"], dtype=np.float32)       # [32, 1, 3]
    cb = np.asarray(inputs["conv_b"], dtype=np.float32)      # [32]
    pw = np.asarray(inputs["proj_w"], dtype=np.float32)      # [512, 16384]
    pb = np.asarray(inputs["proj_b"], dtype=np.float32)      # [512]
    h = np.asarray(inputs["h"]).astype(np.int32)
    r = np.asarray(inputs["r"]).astype(np.int32)
    t = np.asarray(inputs["t"]).astype(np.int32)

    hI = np.ascontiguousarray(h.reshape(NQ, 128).T)
    rI = np.ascontiguousarray(r.reshape(NQ, 128).T)
    tI = np.ascontiguousarray(t.reshape(NQ, 128).T)

    jl = np.arange(JB)
    jl8 = np.arange(8)
    in_maps = []
    for m in range(NCORES):
        band = np.zeros((128, NSEG, JB), np.float32)
        bstub = np.zeros((10, 32), np.float32)
        pwt = np.zeros((JB, NSEG, D), np.float32)
        pstub = np.zeros((KSTUB, D), np.float32)
        cbias = np.zeros((128, CPC), np.float32)
        sbias = np.zeros((32, 1), np.float32)
        for c in range(CPC):
            cg = CPC * m + c
            cbias[:, c] = cb[cg]
            sbias[c * 8:(c + 1) * 8, 0] = cb[cg]
            for k in range(3):
                bstub[jl8 + k, c * 8 + jl8] = w[cg, 0, k]
            for s in range(4):
                cs = c * 4 + s
                for k in range(3):
                    band[jl + k, cs, jl] = w[cg, 0, k]
                pwt[:, cs, :] = pw[:, cg * D + JB * s: cg * D + JB * (s + 1)].T
            pstub[c * 8:(c + 1) * 8, :] = pw[:, cg * D + 504: cg * D + 512].T
        if m == 0:
            pstub[32] = pb
        in_maps.append({
            "ent": ent, "rel": rel, "hI": hI, "tI": tI, "rI": rI,
            "band": np.ascontiguousarray(band.reshape(128, NSEG * JB)).astype(bf),
            "bstub": bstub.astype(bf),
            "pwt": np.ascontiguousarray(pwt.reshape(JB, NSEG * D)).astype(bf),
            "pstub": pstub.astype(bf), "cbias": cbias, "sbias": sbias,
        })
    return in_maps


def _run(inputs, trace=False, tmpdir=None):
    from concourse.bass_utils import run_bass_kernel_spmd

    if "nc" not in _CACHE:
        _CACHE["nc"] = _build_nc()
    nc = _CACHE["nc"]
    in_maps = _host_prep(inputs)
    res = run_bass_kernel_spmd(nc, in_maps, core_ids=list(range(NCORES)),
                               trace=trace, tmpdir=tmpdir)
    total = np.zeros((128, NQ), np.float64)
    for mres in res.results:
        total += mres["out"].astype(np.float64)
    return total.T.reshape(B).astype(np.float32), res


def kernel(**inputs):
    out, _ = _run(inputs, trace=False)
    return out


# revision 11
# speedup vs baseline: 1.0168x; 1.0168x over previous
"""ConvTransE forward on 8 Trainium2 NeuronCores (Bass/Tile) — bf16 PE, v5.

Math shortcut: the reference computes scores = x @ ent.T ([B, 100000]) and
returns scores[i, t[i]]; we only compute out[b] = x[b] . ent[t[b]].
The conv's retained slice [:, :512] depends only on ent[h] and rel[r][:, 0].

Sharding: tensor-parallel over the projection contraction dim (channels).
Core m owns channels [4m, 4m+4).  Every core:
  - gathers ent[h] rows (cast fp32->bf16 in the DMA), rel[r][:, 0] only,
    and ent[t] rows (fp32) via indirect DMA,
  - PE-transposes overlapping 128-wide windows of the gathered rows (bf16),
  - runs the conv as banded matmuls on the PE (bands built on host, bf16),
  - projects its K-slice:  z_m = relu(conv)_m @ proj_w_m^T  (K = 2048/core),
  - emits partial[b] = z_m[b] . ent[t[b]] via a fused multiply+row-sum (fp32).
proj_b rides along as an extra "ones" contraction row on core 0 only.
Host sums the 8 [2048] partials.

v7: bf16 entity table in DRAM (halves gather HBM reads; t-rows
upcast to fp32 in the DMA), pwt tail loads gated behind the first
h-gathers via WAW dummies, chunk 0 interleaves the head of its own
first projection group into its conv phase.
v6: grouped per-btl projection (no per-matmul PSUM bank cycling — that
measured ~20% slower per matmul), conv PSUM tiles hold 2 segments and
relu+bias runs once per pair (ACT/DVE alternating), and each chunk's
4th projection group is deferred into the middle of the NEXT chunk's
conv phase so the conv relu-drain latency is hidden behind projection
matmuls instead of stalling the in-order PE queue. Gathers prefetched
one chunk ahead (h, then rel, then t), index DMAs before weight DMAs,
pwt in two DMAs, stub transposes packed into each btl's transpose tile.
"""

import numpy as np

NE, NRR, D, C, B = 100000, 500, 512, 32, 2048
NCORES = 8
CPC = C // NCORES          # 4 channels per core
NQ = B // 128              # 16 batch tiles of 128
CHUNK_BT = 4               # batch tiles per pipeline chunk
NCHUNK = NQ // CHUNK_BT
NB = CHUNK_BT * 128        # 512 batch columns per chunk
JB = 126                   # conv j-block (126 outputs need a 128-wide input window)
NSEG = 16                  # (c, s) main contraction blocks per core
KSTUB = CPC * 8 + 1        # 33: packed j=504..511 stub rows + ones row
GWW = 640                  # per-btl transpose tile: 4 windows * 128 + stub 128

_CACHE = {}


def _build_nc():
    from contextlib import ExitStack

    import concourse.bass as bass
    import concourse.tile as tile
    from concourse import bacc, mybir
    from concourse.masks import make_identity

    f32 = mybir.dt.float32
    bf16 = mybir.dt.bfloat16
    i32 = mybir.dt.int32
    Alu = mybir.AluOpType

    nc = bacc.Bacc("TRN2", target_bir_lowering=False, debug=False,
                   num_devices=NCORES)

    ent = nc.dram_tensor("ent", [NE, D], bf16, kind="ExternalInput")
    rel = nc.dram_tensor("rel", [NRR, D], f32, kind="ExternalInput")
    hI = nc.dram_tensor("hI", [128, NQ], i32, kind="ExternalInput")
    tI = nc.dram_tensor("tI", [128, NQ], i32, kind="ExternalInput")
    rI = nc.dram_tensor("rI", [128, NQ], i32, kind="ExternalInput")
    band = nc.dram_tensor("band", [128, NSEG * JB], bf16, kind="ExternalInput")
    bstub = nc.dram_tensor("bstub", [10, 32], bf16, kind="ExternalInput")
    pwt = nc.dram_tensor("pwt", [JB, NSEG * D], bf16, kind="ExternalInput")
    pstub = nc.dram_tensor("pstub", [KSTUB, D], bf16, kind="ExternalInput")
    cbias = nc.dram_tensor("cbias", [128, CPC], f32, kind="ExternalInput")
    sbias = nc.dram_tensor("sbias", [32, 1], f32, kind="ExternalInput")
    out = nc.dram_tensor("out", [128, NQ], f32, kind="ExternalOutput")

    with tile.TileContext(nc) as tc, ExitStack() as ctx:
        const = ctx.enter_context(tc.tile_pool(name="const", bufs=1))
        gpad_p = ctx.enter_context(tc.tile_pool(name="gpad", bufs=10))
        v_p = ctx.enter_context(tc.tile_pool(name="vt", bufs=12))
        gw_p = ctx.enter_context(tc.tile_pool(name="gw", bufs=2))
        gws_p = ctx.enter_context(tc.tile_pool(name="gws", bufs=2))
        y_p = ctx.enter_context(tc.tile_pool(name="ym", bufs=2))
        ys_p = ctx.enter_context(tc.tile_pool(name="ys", bufs=2))
        sc_p = ctx.enter_context(tc.tile_pool(name="scr", bufs=2))
        tp_p = ctx.enter_context(tc.tile_pool(name="tp", bufs=2, space="PSUM"))
        yp_p = ctx.enter_context(tc.tile_pool(name="yp", bufs=2, space="PSUM"))
        z_p = ctx.enter_context(tc.tile_pool(name="zp", bufs=2, space="PSUM"))

        # tiny index tables first: the first gathers depend only on these
        hI_sb = const.tile([128, NQ], i32)
        nc.sync.dma_start(hI_sb[:], hI[:])
        rI_sb = const.tile([128, NQ], i32)
        nc.sync.dma_start(rI_sb[:], rI[:])
        ident = const.tile([128, 128], bf16)
        make_identity(nc, ident[:])
        band_sb = const.tile([128, NSEG * JB], bf16)
        nc.sync.dma_start(band_sb[:], band[:])
        pwt_sb = const.tile([JB, NSEG * D], bf16)
        nc.sync.dma_start(pwt_sb[:, 0:4 * D], pwt[:, 0:4 * D])
        bstub_sb = const.tile([10, 32], bf16)
        nc.sync.dma_start(bstub_sb[:], bstub[:])
        cb_sb = const.tile([128, CPC], f32)
        nc.sync.dma_start(cb_sb[:], cbias[:])
        sb_sb = const.tile([32, 1], f32)
        nc.sync.dma_start(sb_sb[:], sbias[:])
        tI_sb = const.tile([128, NQ], i32)
        nc.sync.dma_start(tI_sb[:], tI[:])
        pstub_sb = const.tile([KSTUB, D], bf16)
        nc.sync.dma_start(pstub_sb[:], pstub[:])
        out_sb = const.tile([128, NQ], f32)

        def emit_gathers(c):
            gpads, vts = [], []
            for btl in range(CHUNK_BT):
                q = c * CHUNK_BT + btl
                gpad = gpad_p.tile([128, D + 2], bf16, name="gpad")
                nc.vector.memset(gpad[:, 0:1], 0.0)
                nc.gpsimd.indirect_dma_start(
                    out=gpad[:, 1:D + 1], out_offset=None, in_=ent[:],
                    in_offset=bass.IndirectOffsetOnAxis(
                        ap=hI_sb[:, q:q + 1], axis=0))
                gpads.append(gpad)
            for btl in range(CHUNK_BT):
                q = c * CHUNK_BT + btl
                nc.gpsimd.indirect_dma_start(
                    out=gpads[btl][:, D + 1:D + 2], out_offset=None,
                    in_=rel[:, 0:1],
                    in_offset=bass.IndirectOffsetOnAxis(
                        ap=rI_sb[:, q:q + 1], axis=0))
            for btl in range(CHUNK_BT):
                q = c * CHUNK_BT + btl
                vt = v_p.tile([128, D], f32, name="vt")
                nc.gpsimd.indirect_dma_start(
                    out=vt[:], out_offset=None, in_=ent[:],
                    in_offset=bass.IndirectOffsetOnAxis(
                        ap=tI_sb[:, q:q + 1], axis=0))
                vts.append(vt)
            return gpads, vts

        def emit_proj_head(q, ym, hi):
            z = z_p.tile([128, D], f32, name="zt")
            b = q % CHUNK_BT
            for i in range(hi):
                nc.tensor.matmul(
                    z[:], ym[:, i * NB + b * 128:i * NB + (b + 1) * 128],
                    pwt_sb[:, i * D:(i + 1) * D],
                    start=(i == 0), stop=False)
            return z

        def emit_proj(q, ym, ystub, vt, z=None, lo=0):
            b = q % CHUNK_BT
            if z is None:
                z = z_p.tile([128, D], f32, name="zt")
            for i in range(lo, NSEG):
                nc.tensor.matmul(
                    z[:], ym[:, i * NB + b * 128:i * NB + (b + 1) * 128],
                    pwt_sb[:, i * D:(i + 1) * D],
                    start=(i == 0), stop=False)
            nc.tensor.matmul(z[:], ystub[:, b * 128:(b + 1) * 128],
                             pstub_sb[:], start=False, stop=True)
            scr = sc_p.tile([128, D], f32)
            nc.vector.scalar_tensor_tensor(
                out=scr[:], in0=z[:], scalar=1.0, in1=vt[:],
                op0=Alu.mult, op1=Alu.mult,
                accum_out=out_sb[:, q:q + 1])

        pending = emit_gathers(0)
        # gate the pwt tail loads on the first h-gather landing so the
        # chunk-0 gathers aren't starved of HBM by the 2MB weight stream
        nc.vector.tensor_copy(pwt_sb[0:1, 4 * D:4 * D + 1],
                              pending[0][0][0:1, 1:2])
        nc.sync.dma_start(pwt_sb[:, 4 * D:10 * D], pwt[:, 4 * D:10 * D])
        nc.vector.tensor_copy(pwt_sb[0:1, 10 * D:10 * D + 1],
                              pending[0][1][0:1, 1:2])
        nc.sync.dma_start(pwt_sb[:, 10 * D:], pwt[:, 10 * D:])
        deferred = None    # (q, ym, ystub, vt) for the previous chunk's b3
        for chunk in range(NCHUNK):
            gpads, vts = pending
            if chunk + 1 < NCHUNK:
                pending = emit_gathers(chunk + 1)

            # per-btl: 4 main window transposes + stub transpose into one
            # PSUM tile, one main copy into btl-major gw, one stub copy
            gw = gw_p.tile([128, CHUNK_BT * D], bf16)
            gwv = gw[:].rearrange("p (b s c) -> p b s c",
                                  b=CHUNK_BT, s=4, c=128)
            gws = gws_p.tile([10, NB], bf16)
            for btl in range(CHUNK_BT):
                gpad = gpads[btl]
                tp = tp_p.tile([128, GWW], bf16)
                for s in range(4):
                    nc.tensor.transpose(tp[:, s * 128:(s + 1) * 128],
                                        gpad[:, JB * s:JB * s + 128], ident[:])
                nc.tensor.transpose(tp[0:10, 512:640],
                                    gpad[:, 4 * JB:D + 2], ident[:])
                nc.vector.tensor_copy(gw[:, btl * D:(btl + 1) * D],
                                      tp[:, 0:512])
                nc.vector.tensor_copy(gws[:, btl * 128:(btl + 1) * 128],
                                      tp[0:10, 512:640])

            ym = y_p.tile([JB, NSEG * NB], bf16)
            ystub = ys_p.tile([KSTUB, NB], bf16)
            nc.vector.memset(ystub[32:33, :], 1.0)

            # conv: 2 segments per PSUM tile, one relu+bias op per pair
            def conv_pair(k):
                yp = yp_p.tile([JB, 2 * NB], f32, name="yp")
                for j in range(2):
                    cs = 2 * k + j
                    s4 = cs % 4
                    nc.tensor.matmul(yp[:, j * NB:(j + 1) * NB],
                                     band_sb[:, cs * JB:(cs + 1) * JB],
                                     gwv[:, :, s4, :], start=True, stop=True)
                c4 = (2 * k) // 4
                if k % 2 == 0:
                    nc.scalar.activation(
                        ym[:, 2 * k * NB:(2 * k + 2) * NB], yp[:],
                        mybir.ActivationFunctionType.Relu,
                        bias=cb_sb[0:JB, c4:c4 + 1])
                else:
                    nc.vector.tensor_scalar(ym[:, 2 * k * NB:(2 * k + 2) * NB],
                                            yp[:], cb_sb[0:JB, c4:c4 + 1],
                                            0.0, Alu.add, Alu.max)

            for k in range(4):
                conv_pair(k)
            # previous chunk's 4th projection group fills the PE while this
            # chunk's first relus drain; chunk 0 uses the head of its own
            # first group instead
            z0 = None
            if deferred is not None:
                emit_proj(*deferred)
            else:
                z0 = emit_proj_head(chunk * CHUNK_BT, ym, 8)
            for k in range(4, NSEG // 2):
                conv_pair(k)
            yps = yp_p.tile([JB, 2 * NB], f32, name="yp")
            nc.tensor.matmul(yps[0:32, 0:NB], bstub_sb[:], gws[:],
                             start=True, stop=True)
            nc.scalar.activation(ystub[0:32, :], yps[0:32, 0:NB],
                                 mybir.ActivationFunctionType.Relu,
                                 bias=sb_sb[:, 0:1])

            first = 0
            if z0 is not None:
                emit_proj(chunk * CHUNK_BT, ym, ystub, vts[0], z=z0, lo=8)
                first = 1
            for btl in range(first, CHUNK_BT - 1):
                emit_proj(chunk * CHUNK_BT + btl, ym, ystub, vts[btl])
            deferred = (chunk * CHUNK_BT + 3, ym, ystub, vts[3])

        emit_proj(*deferred)
        nc.sync.dma_start(out[:], out_sb[:])
    nc.finalize()
    return nc


def _host_prep(inputs):
    """Per-core input dicts from the full problem inputs."""
    import ml_dtypes

    bf = ml_dtypes.bfloat16
    ent = np.ascontiguousarray(
        np.asarray(inputs["ent"], dtype=np.float32).astype(bf))
    rel = np.ascontiguousarray(np.asarray(inputs["rel"], dtype=np.float32))
    w = np.asarray(inputs["conv_w"], dtype=np.float32)       # [32, 1, 3]
    cb = np.asarray(inputs["conv_b"], dtype=np.float32)      # [32]
    pw = np.asarray(inputs["proj_w"], dtype=np.float32)      # [512, 16384]
    pb = np.asarray(inputs["proj_b"], dtype=np.float32)      # [512]
    h = np.asarray(inputs["h"]).astype(np.int32)
    r = np.asarray(inputs["r"]).astype(np.int32)
    t = np.asarray(inputs["t"]).astype(np.int32)

    hI = np.ascontiguousarray(h.reshape(NQ, 128).T)
    rI = np.ascontiguousarray(r.reshape(NQ, 128).T)
    tI = np.ascontiguousarray(t.reshape(NQ, 128).T)

    jl = np.arange(JB)
    jl8 = np.arange(8)
    in_maps = []
    for m in range(NCORES):
        band = np.zeros((128, NSEG, JB), np.float32)
        bstub = np.zeros((10, 32), np.float32)
        pwt = np.zeros((JB, NSEG, D), np.float32)
        pstub = np.zeros((KSTUB, D), np.float32)
        cbias = np.zeros((128, CPC), np.float32)
        sbias = np.zeros((32, 1), np.float32)
        for c in range(CPC):
            cg = CPC * m + c
            cbias[:, c] = cb[cg]
            sbias[c * 8:(c + 1) * 8, 0] = cb[cg]
            for k in range(3):
                bstub[jl8 + k, c * 8 + jl8] = w[cg, 0, k]
            for s in range(4):
                cs = c * 4 + s
                for k in range(3):
                    band[jl + k, cs, jl] = w[cg, 0, k]
                pwt[:, cs, :] = pw[:, cg * D + JB * s: cg * D + JB * (s + 1)].T
            pstub[c * 8:(c + 1) * 8, :] = pw[:, cg * D + 504: cg * D + 512].T
        if m == 0:
            pstub[32] = pb
        in_maps.append({
            "ent": ent, "rel": rel, "hI": hI, "tI": tI, "rI": rI,
            "band": np.ascontiguousarray(band.reshape(128, NSEG * JB)).astype(bf),
            "bstub": bstub.astype(bf),
            "pwt": np.ascontiguousarray(pwt.reshape(JB, NSEG * D)).astype(bf),
            "pstub": pstub.astype(bf), "cbias": cbias, "sbias": sbias,
        })
    return in_maps


def _run(inputs, trace=False, tmpdir=None):
    from concourse.bass_utils import run_bass_kernel_spmd

    if "nc" not in _CACHE:
        _CACHE["nc"] = _build_nc()
    nc = _CACHE["nc"]
    in_maps = _host_prep(inputs)
    res = run_bass_kernel_spmd(nc, in_maps, core_ids=list(range(NCORES)),
                               trace=trace, tmpdir=tmpdir)
    total = np.zeros((128, NQ), np.float64)
    for mres in res.results:
        total += mres["out"].astype(np.float64)
    return total.T.reshape(B).astype(np.float32), res


def kernel(**inputs):
    out, _ = _run(inputs, trace=False)
    return out
